# revision 6
# baseline (speedup 1.0000x reference)
"""Fused multi-head attention (LN + QKV + softmax + out-proj) for TRN2,
sharded over 8 NeuronCores: batch (4) x head-group (2 groups of 6 heads).

Each core computes, for its (batch, head-group) shard:
    xn = LayerNorm(x[b]) ; q/k/v head projections (gamma+1 and the 1/sqrt(dh)
    scale are folded into the weights host-side) ; scores S^T = K^T.T @ Q^T
    per 128-key tile ; P^T = exp(S^T) ; attn^T = [V|1].T @ P^T accumulated
    over key tiles (row 64 = softmax denominators) ; normalize ; partial
    out = attn^T.T @ WoT for its 6 heads.
The two cores sharing a batch produce additive partials; the host sums them.

All matmul operands are bf16 (f32 PSUM accumulation); LN stats and softmax
normalization are f32.
"""
import numpy as np

import concourse.bass as bass
import concourse.bacc as bacc
import concourse.tile as tile
from concourse import mybir
from concourse.bass_utils import run_bass_kernel_spmd

F32 = mybir.dt.float32
BF16 = mybir.dt.bfloat16
AF = mybir.ActivationFunctionType
ALU = mybir.AluOpType

B, N, DIM, H, DH = 4, 2048, 768, 12, 64
NCORES = 8
NH = 6            # heads per core
NP = 3            # head pairs per core
HCOLS = NH * DH   # 384


def build_graph(n=N, dim=DIM, num_devices=NCORES):
    """Build the per-core graph. n/dim shrinkable for simulator testing."""
    nt = n // 128        # token/key tiles
    ncdm = dim // 128    # dmodel chunks
    nqc = n // 512       # 512-wide query/token column chunks
    hcols = HCOLS

    nc = bacc.Bacc("TRN2", target_bir_lowering=False, debug=False,
                   num_devices=num_devices)
    x = nc.dram_tensor("x", [n, dim], F32, kind="ExternalInput").ap()
    wqt = nc.dram_tensor("wqt", [dim, hcols], BF16, kind="ExternalInput").ap()
    wkt = nc.dram_tensor("wkt", [dim, hcols], BF16, kind="ExternalInput").ap()
    wvt = nc.dram_tensor("wvt", [dim, hcols], BF16, kind="ExternalInput").ap()
    wot = nc.dram_tensor("wot", [hcols, dim], BF16, kind="ExternalInput").ap()
    ident = nc.dram_tensor("ident", [128, 128], BF16, kind="ExternalInput").ap()
    out = nc.dram_tensor("out", [n, dim], F32, kind="ExternalOutput").ap()

    with tile.TileContext(nc) as tc:
        _body(tc, x, wqt, wkt, wvt, wot, ident, out, n, dim, nt, ncdm, nqc)
    nc.compile()
    return nc


def _body(tc, x, wqt, wkt, wvt, wot, ident, out, n, dim, nt, ncdm, nqc):
    nc = tc.nc
    from contextlib import ExitStack
    with ExitStack() as ctx:
        consts = ctx.enter_context(tc.tile_pool(name="consts", bufs=1))
        sb = ctx.enter_context(tc.tile_pool(name="sb", bufs=1))
        xpool = ctx.enter_context(tc.tile_pool(name="xp", bufs=3))
        small = ctx.enter_context(tc.tile_pool(name="small", bufs=4))
        ppool = ctx.enter_context(tc.tile_pool(name="pp", bufs=3))
        rbpool = ctx.enter_context(tc.tile_pool(name="rb", bufs=2))
        oddp = ctx.enter_context(tc.tile_pool(name="odd", bufs=2))
        ps = ctx.enter_context(tc.tile_pool(name="ps", bufs=2, space="PSUM"))
        pvp = ctx.enter_context(tc.tile_pool(name="pv", bufs=1, space="PSUM"))

        # constants + weights
        eps_sb = consts.tile([128, 1], F32, tag="eps")
        nc.vector.memset(eps_sb, 1e-5)
        id_sb = consts.tile([128, 128], BF16, tag="id")
        nc.sync.dma_start(out=id_sb, in_=ident)
        wq_sb = consts.tile([128, ncdm, HCOLS], BF16, tag="wq")
        nc.sync.dma_start(out=wq_sb, in_=wqt.rearrange("(c p) m -> p c m", p=128))
        wk_sb = consts.tile([128, ncdm, HCOLS], BF16, tag="wk")
        nc.sync.dma_start(out=wk_sb, in_=wkt.rearrange("(c p) m -> p c m", p=128))
        wv_sb = consts.tile([128, ncdm, HCOLS], BF16, tag="wv")
        nc.sync.dma_start(out=wv_sb, in_=wvt.rearrange("(c p) m -> p c m", p=128))
        wo_sb = consts.tile([128, NP, dim], BF16, tag="wo")
        nc.sync.dma_start(out=wo_sb, in_=wot.rearrange("(c p) m -> p c m", p=128))

        # persistent activations
        xnT = sb.tile([128, ncdm, n], BF16, tag="xnT")
        qt_sb = sb.tile([128, NP, n], BF16, tag="qt")
        kt_sb = sb.tile([128, NP, n], BF16, tag="kt")
        v_sb = sb.tile([128, NH, nt, DH + 1], BF16, tag="v")
        att_sb = sb.tile([128, NP, n], BF16, tag="att")

        nc.vector.memset(v_sb[:, :, :, DH:DH + 1], 1.0)

        x3 = x.rearrange("(t p) d -> t p d", p=128)
        out3 = out.rearrange("(t p) d -> t p d", p=128)

        # ---- LayerNorm + transpose to xnT ----
        for tt in range(nt):
            xt = xpool.tile([128, dim], F32, tag="x")
            nc.sync.dma_start(out=xt, in_=x3[tt])
            ngr = dim // 256
            stats = small.tile([128, ngr, 6], F32, tag="stats")
            for g in range(ngr):
                nc.vector.bn_stats(out=stats[:, g, :], in_=xt[:, g * 256:(g + 1) * 256])
            mv = small.tile([128, 2], F32, tag="mv")
            nc.vector.bn_aggr(out=mv, in_=stats)
            sq = small.tile([128, 1], F32, tag="sq")
            nc.scalar.activation(out=sq, in_=mv[:, 1:2], func=AF.Sqrt, bias=eps_sb)
            rstd = small.tile([128, 1], F32, tag="rstd")
            nc.vector.reciprocal(out=rstd, in_=sq)
            xn = xpool.tile([128, dim], BF16, tag="xn")
            nc.vector.tensor_scalar(out=xn, in0=xt, scalar1=mv[:, 0:1],
                                    scalar2=rstd, op0=ALU.subtract, op1=ALU.mult)
            for c in range(ncdm):
                pt = ps.tile([128, 128], BF16, tag="ps")
                nc.tensor.transpose(pt, xn[:, c * 128:(c + 1) * 128], id_sb)
                nc.vector.tensor_copy(out=xnT[:, c, tt * 128:(tt + 1) * 128], in_=pt)

        # ---- Q^T / K^T / V projections (per head pair) ----
        for i in range(NP):
            for wsb, dst in ((wq_sb, qt_sb), (wk_sb, kt_sb)):
                for cc in range(nqc):
                    pst = ps.tile([128, 512], F32, tag="ps")
                    for c in range(ncdm):
                        nc.tensor.matmul(pst, wsb[:, c, i * 128:(i + 1) * 128],
                                         xnT[:, c, cc * 512:(cc + 1) * 512],
                                         start=(c == 0), stop=(c == ncdm - 1))
                    nc.vector.tensor_copy(out=dst[:, i, cc * 512:(cc + 1) * 512], in_=pst)
            for tt in range(nt):
                pst = ps.tile([128, 128], F32, tag="ps")
                for c in range(ncdm):
                    nc.tensor.matmul(pst, xnT[:, c, tt * 128:(tt + 1) * 128],
                                     wv_sb[:, c, i * 128:(i + 1) * 128],
                                     start=(c == 0), stop=(c == ncdm - 1))
                nc.vector.tensor_copy(out=v_sb[:, 2 * i, tt, 0:DH], in_=pst[:, 0:DH])
                nc.vector.tensor_copy(out=v_sb[:, 2 * i + 1, tt, 0:DH], in_=pst[:, DH:2 * DH])

        # ---- attention per head ----
        for h in range(NH):
            i, s = h // 2, h % 2
            po = s * 64
            pv = pvp.tile([65, n], F32, tag="pv")
            for kt in range(nt):
                p_t = ppool.tile([128, n], BF16, tag="p")
                for half in range(max(1, n // 1024)):
                    w = min(1024, n)
                    sc = ps.tile([128, w], F32, tag="ps")
                    for qq in range(w // 512):
                        q0 = half * 1024 + qq * 512
                        nc.tensor.matmul(sc[:, qq * 512:(qq + 1) * 512],
                                         kt_sb[po:po + 64, i, kt * 128:(kt + 1) * 128],
                                         qt_sb[po:po + 64, i, q0:q0 + 512])
                    nc.scalar.activation(out=p_t[:, half * w:(half + 1) * w], in_=sc,
                                         func=AF.Exp)
                for qc in range(nqc):
                    nc.tensor.matmul(pv[:, qc * 512:(qc + 1) * 512],
                                     v_sb[:, h, kt, :],
                                     p_t[:, qc * 512:(qc + 1) * 512],
                                     start=(kt == 0), stop=(kt == nt - 1))
            # normalize by softmax denominators (row 64), land in att_sb
            srow = rbpool.tile([1, n], F32, tag="srow")
            nc.vector.tensor_copy(out=srow, in_=pv[64:65, :])
            rrow = rbpool.tile([1, n], F32, tag="srow")
            nc.vector.reciprocal(out=rrow, in_=srow)
            rc = rbpool.tile([64, n], F32, tag="rb")
            nc.gpsimd.partition_broadcast(rc, rrow)
            if s == 0:
                nc.vector.tensor_mul(out=att_sb[0:64, i, :], in0=pv[0:64, :], in1=rc)
            else:
                tmp = oddp.tile([64, n], BF16, tag="odd")
                nc.vector.tensor_mul(out=tmp, in0=pv[0:64, :], in1=rc)
                nc.sync.dma_start(out=att_sb[64:128, i, :], in_=tmp)

        # ---- output projection (partial over this core's heads) ----
        for tt in range(nt):
            po_t = ps.tile([128, dim], F32, tag="ps")
            for c in range(NP):
                lhsT = att_sb[:, c, tt * 128:(tt + 1) * 128]
                for o0 in range(0, dim, 512):
                    o1 = min(o0 + 512, dim)
                    nc.tensor.matmul(po_t[:, o0:o1], lhsT, wo_sb[:, c, o0:o1],
                                     start=(c == 0), stop=(c == NP - 1))
            ot = xpool.tile([128, dim], F32, tag="ot")
            nc.vector.tensor_copy(out=ot, in_=po_t)
            nc.sync.dma_start(out=out3[tt], in_=ot)


_NC_CACHE = {}


def _get_nc():
    if "nc" not in _NC_CACHE:
        _NC_CACHE["nc"] = build_graph()
    return _NC_CACHE["nc"]


def make_in_maps(x, gamma, Wq, Wk, Wv, Wo):
    """Host-side sharding: core c -> batch c//2, head-group c%2."""
    import ml_dtypes
    bf16 = ml_dtypes.bfloat16
    g = (np.asarray(gamma, np.float32) + 1.0)
    scale = DH ** -0.5
    Wq_eff = np.asarray(Wq, np.float32) * g[None, :] * scale
    Wk_eff = np.asarray(Wk, np.float32) * g[None, :]
    Wv_eff = np.asarray(Wv, np.float32)
    Wo_eff = np.asarray(Wo, np.float32)
    ident = np.eye(128, dtype=bf16)
    hg_maps = []
    for hg in range(2):
        r0, r1 = hg * HCOLS, (hg + 1) * HCOLS
        hg_maps.append({
            "wqt": np.ascontiguousarray(Wq_eff[r0:r1, :].T).astype(bf16),
            "wkt": np.ascontiguousarray(Wk_eff[r0:r1, :].T).astype(bf16),
            "wvt": np.ascontiguousarray(Wv_eff[r0:r1, :].T).astype(bf16),
            "wot": np.ascontiguousarray(Wo_eff[:, r0:r1].T).astype(bf16),
            "ident": ident,
        })
    in_maps = []
    for c in range(NCORES):
        b, hg = c // 2, c % 2
        m = dict(hg_maps[hg])
        m["x"] = np.ascontiguousarray(np.asarray(x, np.float32)[b])
        in_maps.append(m)
    return in_maps


def _run(inputs, trace=False, trace_kwargs=None):
    nc = _get_nc()
    in_maps = make_in_maps(**inputs)
    res = run_bass_kernel_spmd(nc, in_maps, core_ids=list(range(NCORES)),
                               trace=trace, **(trace_kwargs or {}))
    out = np.empty((B, N, DIM), np.float32)
    for b in range(B):
        out[b] = res.results[2 * b]["out"] + res.results[2 * b + 1]["out"]
    return out, res


def kernel(x, gamma, Wq, Wk, Wv, Wo):
    out, _ = _run(dict(x=x, gamma=gamma, Wq=Wq, Wk=Wk, Wv=Wv, Wo=Wo))
    return out


# revision 8
# speedup vs baseline: 1.1222x; 1.1222x over previous
"""Fused multi-head attention (LN + QKV + softmax + out-proj) for TRN2,
sharded over 8 NeuronCores: batch (4) x head-group (2 groups of 6 heads).

Per core, for its (batch, head-group) shard:
    xn = LayerNorm(x[b])      (gamma+1 and 1/sqrt(dh) folded into weights)
    Q^T,K^T = W @ xn^T        (head-pair-packed, bf16 matmuls, f32 PSUM)
    V = xn @ Wv               (augmented with a ones column)
    per head, per query-half, per 128-key tile:
        S^T = K^T.T Q^T ; P^T = exp(S^T - C)   (ACT, fp8 out)
        attn^T[65,q] += [V|1].T P^T            (fp8 DoubleRow over key-tile pairs;
                                                row 64 = softmax denominators)
    normalize by row 64, out partial = attn^T.T @ WoT   (bf16 partial out)
Host sums the two partials per batch.
"""
import numpy as np

import concourse.bass as bass
import concourse.bacc as bacc
import concourse.tile as tile
from concourse import mybir
from concourse.bass_utils import run_bass_kernel_spmd

F32 = mybir.dt.float32
BF16 = mybir.dt.bfloat16
FP8 = mybir.dt.float8e4
AF = mybir.ActivationFunctionType
ALU = mybir.AluOpType

B, N, DIM, H, DH = 4, 2048, 768, 12, 64
NCORES = 8
NH = 6            # heads per core
NP = 3            # head pairs per core
HCOLS = NH * DH   # 384
EXP_SHIFT = 3.0   # exp(s - C): keeps fp8 P in range; cancels in softmax

USE_FP8 = False
OUT_BF16 = True


def build_graph(n=N, dim=DIM, num_devices=NCORES, use_fp8=USE_FP8,
                out_bf16=OUT_BF16):
    nt = n // 128        # token/key tiles
    ncdm = dim // 128    # dmodel chunks
    qhw = min(n, 1024)   # query-half width
    nqh = n // qhw

    nc = bacc.Bacc("TRN2", target_bir_lowering=False, debug=False,
                   num_devices=num_devices)
    x = nc.dram_tensor("x", [n, dim], F32, kind="ExternalInput").ap()
    wqt = nc.dram_tensor("wqt", [dim, HCOLS], BF16, kind="ExternalInput").ap()
    wkt = nc.dram_tensor("wkt", [dim, HCOLS], BF16, kind="ExternalInput").ap()
    wvt = nc.dram_tensor("wvt", [dim, HCOLS], BF16, kind="ExternalInput").ap()
    wot = nc.dram_tensor("wot", [HCOLS, dim], BF16, kind="ExternalInput").ap()
    ident = nc.dram_tensor("ident", [128, 128], BF16, kind="ExternalInput").ap()
    odt = BF16 if out_bf16 else F32
    out = nc.dram_tensor("out", [n, dim], odt, kind="ExternalOutput").ap()

    with tile.TileContext(nc) as tc:
        _body(tc, x, wqt, wkt, wvt, wot, ident, out,
              n, dim, nt, ncdm, qhw, nqh, use_fp8)
    nc.compile()
    return nc


def _body(tc, x, wqt, wkt, wvt, wot, ident, out,
          n, dim, nt, ncdm, qhw, nqh, use_fp8):
    nc = tc.nc
    from contextlib import ExitStack
    pdt = FP8 if use_fp8 else BF16
    with ExitStack() as ctx:
        consts = ctx.enter_context(tc.tile_pool(name="consts", bufs=1))
        sb = ctx.enter_context(tc.tile_pool(name="sb", bufs=1))
        xpool = ctx.enter_context(tc.tile_pool(name="xp", bufs=4))
        small = ctx.enter_context(tc.tile_pool(name="small", bufs=4))
        ppool = ctx.enter_context(tc.tile_pool(name="pp", bufs=3))
        rbpool = ctx.enter_context(tc.tile_pool(name="rb", bufs=3))
        oddp = ctx.enter_context(tc.tile_pool(name="odd", bufs=2))
        otp = ctx.enter_context(tc.tile_pool(name="ot", bufs=4))

        # constants + weights
        eps_sb = consts.tile([128, 1], F32, tag="eps")
        nc.vector.memset(eps_sb, 1e-5)
        shift_sb = consts.tile([128, 1], F32, tag="shift")
        nc.vector.memset(shift_sb, -EXP_SHIFT)
        id_sb = consts.tile([128, 128], BF16, tag="id")
        nc.sync.dma_start(out=id_sb, in_=ident)
        wq_sb = consts.tile([128, ncdm, HCOLS], BF16, tag="wq")
        nc.sync.dma_start(out=wq_sb, in_=wqt.rearrange("(c p) m -> p c m", p=128))
        wk_sb = consts.tile([128, ncdm, HCOLS], BF16, tag="wk")
        nc.sync.dma_start(out=wk_sb, in_=wkt.rearrange("(c p) m -> p c m", p=128))
        wv_sb = consts.tile([128, ncdm, HCOLS], BF16, tag="wv")
        nc.sync.dma_start(out=wv_sb, in_=wvt.rearrange("(c p) m -> p c m", p=128))
        wo_sb = consts.tile([128, NP, dim], BF16, tag="wo")
        nc.sync.dma_start(out=wo_sb, in_=wot.rearrange("(c p) m -> p c m", p=128))

        # persistent activations
        xnT = sb.tile([128, ncdm, n], BF16, tag="xnT")
        qt_sb = sb.tile([128, NP, n], BF16, tag="qt")
        kt_sb = sb.tile([128, NP, n], BF16, tag="kt")
        if use_fp8:
            v_sb = sb.tile([128, NH, nt // 2, 2, DH + 1], pdt, tag="v")
            nc.vector.memset(v_sb[:, :, :, :, DH:DH + 1], 1.0)
        else:
            v_sb = sb.tile([128, NH, nt, DH + 1], pdt, tag="v")
            nc.vector.memset(v_sb[:, :, :, DH:DH + 1], 1.0)
        att_sb = sb.tile([128, NP, n], BF16, tag="att")

        x3 = x.rearrange("(t p) d -> t p d", p=128)
        out3 = out.rearrange("(t p) d -> t p d", p=128)

        # ---- phase 1: LayerNorm + transpose + Q/K/V projections ----
        with tc.tile_pool(name="psA", bufs=6, space="PSUM") as psA:
            for tt in range(nt):
                xt = xpool.tile([128, dim], F32, tag="x")
                nc.sync.dma_start(out=xt, in_=x3[tt])
                ngr = dim // 256
                stats = small.tile([128, ngr, 6], F32, tag="stats")
                for g in range(ngr):
                    nc.vector.bn_stats(out=stats[:, g, :],
                                       in_=xt[:, g * 256:(g + 1) * 256])
                mv = small.tile([128, 2], F32, tag="mv")
                nc.vector.bn_aggr(out=mv, in_=stats)
                sq = small.tile([128, 1], F32, tag="sq")
                nc.scalar.activation(out=sq, in_=mv[:, 1:2], func=AF.Sqrt,
                                     bias=eps_sb)
                rstd = small.tile([128, 1], F32, tag="rstd")
                nc.vector.reciprocal(out=rstd, in_=sq)
                xn = xpool.tile([128, dim], BF16, tag="xn")
                nc.vector.tensor_scalar(out=xn, in0=xt, scalar1=mv[:, 0:1],
                                        scalar2=rstd, op0=ALU.subtract,
                                        op1=ALU.mult)
                for c in range(ncdm):
                    pt = psA.tile([128, 128], BF16, tag="psA")
                    nc.tensor.transpose(pt, xn[:, c * 128:(c + 1) * 128], id_sb)
                    nc.vector.tensor_copy(out=xnT[:, c, tt * 128:(tt + 1) * 128],
                                          in_=pt)

            for i in range(NP):
                for wsb, dst in ((wq_sb, qt_sb), (wk_sb, kt_sb)):
                    for cc in range(n // 512):
                        pst = psA.tile([128, 512], F32, tag="psA")
                        for c in range(ncdm):
                            nc.tensor.matmul(pst, wsb[:, c, i * 128:(i + 1) * 128],
                                             xnT[:, c, cc * 512:(cc + 1) * 512],
                                             start=(c == 0), stop=(c == ncdm - 1))
                        nc.vector.tensor_copy(out=dst[:, i, cc * 512:(cc + 1) * 512],
                                              in_=pst)
                for tt in range(nt):
                    pst = psA.tile([128, 128], F32, tag="psA")
                    for c in range(ncdm):
                        nc.tensor.matmul(pst, xnT[:, c, tt * 128:(tt + 1) * 128],
                                         wv_sb[:, c, i * 128:(i + 1) * 128],
                                         start=(c == 0), stop=(c == ncdm - 1))
                    if use_fp8:
                        va = v_sb[:, 2 * i, tt // 2, tt % 2, 0:DH]
                        vb = v_sb[:, 2 * i + 1, tt // 2, tt % 2, 0:DH]
                    else:
                        va = v_sb[:, 2 * i, tt, 0:DH]
                        vb = v_sb[:, 2 * i + 1, tt, 0:DH]
                    nc.vector.tensor_copy(out=va, in_=pst[:, 0:DH])
                    nc.vector.tensor_copy(out=vb, in_=pst[:, DH:2 * DH])

        # ---- phase 2: attention ----
        with tc.tile_pool(name="psS", bufs=2, space="PSUM") as psS, \
             tc.tile_pool(name="psV", bufs=2, space="PSUM") as psV:
            for h in range(NH):
                i, s = h // 2, h % 2
                po = s * 64
                for qh in range(nqh):
                    q0 = qh * qhw
                    pv = psV.tile([65, qhw], F32, tag="pv")
                    for kt in range(nt):
                        if use_fp8:
                            if kt % 2 == 0:
                                p_t = ppool.tile([128, 2, qhw], pdt, tag="p")
                            p_dst = p_t[:, kt % 2, :]
                        else:
                            p_t = ppool.tile([128, qhw], pdt, tag="p")
                            p_dst = p_t
                        sc = psS.tile([128, qhw], F32, tag="sc")
                        for qq in range(qhw // 512):
                            nc.tensor.matmul(
                                sc[:, qq * 512:(qq + 1) * 512],
                                kt_sb[po:po + 64, i, kt * 128:(kt + 1) * 128],
                                qt_sb[po:po + 64, i, q0 + qq * 512:q0 + (qq + 1) * 512])
                        nc.scalar.activation(out=p_dst, in_=sc, func=AF.Exp,
                                             bias=shift_sb)
                        if use_fp8:
                            if kt % 2 == 1:
                                for qq in range(qhw // 512):
                                    nc.tensor.matmul(
                                        pv[:, qq * 512:(qq + 1) * 512],
                                        v_sb[:, h, kt // 2, :, :],
                                        p_t[:, :, qq * 512:(qq + 1) * 512],
                                        start=(kt == 1), stop=(kt == nt - 1),
                                        perf_mode=mybir.MatmulPerfMode.DoubleRow)
                        else:
                            for qq in range(qhw // 512):
                                nc.tensor.matmul(
                                    pv[:, qq * 512:(qq + 1) * 512],
                                    v_sb[:, h, kt, :],
                                    p_t[:, qq * 512:(qq + 1) * 512],
                                    start=(kt == 0), stop=(kt == nt - 1))
                    # normalize by softmax denominators (row 64)
                    srow = rbpool.tile([1, qhw], F32, tag="srow")
                    nc.vector.tensor_copy(out=srow, in_=pv[64:65, :])
                    rrow = rbpool.tile([1, qhw], F32, tag="srow")
                    nc.vector.reciprocal(out=rrow, in_=srow)
                    rc = rbpool.tile([64, qhw], F32, tag="rb")
                    nc.gpsimd.partition_broadcast(rc, rrow)
                    if s == 0:
                        nc.vector.tensor_mul(out=att_sb[0:64, i, q0:q0 + qhw],
                                             in0=pv[0:64, :], in1=rc)
                    else:
                        tmp = oddp.tile([64, qhw], BF16, tag="odd")
                        nc.vector.tensor_mul(out=tmp, in0=pv[0:64, :], in1=rc)
                        nc.sync.dma_start(out=att_sb[64:128, i, q0:q0 + qhw],
                                          in_=tmp)

        # ---- phase 3: output projection ----
        with tc.tile_pool(name="psO", bufs=3, space="PSUM") as psO:
            for tt in range(nt):
                po_t = psO.tile([128, dim], F32, tag="psO")
                for c in range(NP):
                    lhsT = att_sb[:, c, tt * 128:(tt + 1) * 128]
                    for o0 in range(0, dim, 512):
                        o1 = min(o0 + 512, dim)
                        nc.tensor.matmul(po_t[:, o0:o1], lhsT, wo_sb[:, c, o0:o1],
                                         start=(c == 0), stop=(c == NP - 1))
                ot = otp.tile([128, dim], out.dtype, tag="ot")
                if tt % 2 == 0:
                    nc.vector.tensor_copy(out=ot, in_=po_t)
                else:
                    nc.scalar.copy(out=ot, in_=po_t)
                nc.sync.dma_start(out=out3[tt], in_=ot)


_NC_CACHE = {}


def _get_nc():
    if "nc" not in _NC_CACHE:
        _NC_CACHE["nc"] = build_graph()
    return _NC_CACHE["nc"]


def make_in_maps(x, gamma, Wq, Wk, Wv, Wo):
    """Host-side sharding: core c -> batch c//2, head-group c%2."""
    import ml_dtypes
    bf16 = ml_dtypes.bfloat16
    g = (np.asarray(gamma, np.float32) + 1.0)
    scale = DH ** -0.5
    Wq_eff = np.asarray(Wq, np.float32) * g[None, :] * scale
    Wk_eff = np.asarray(Wk, np.float32) * g[None, :]
    Wv_eff = np.asarray(Wv, np.float32)
    Wo_eff = np.asarray(Wo, np.float32)
    ident = np.eye(128, dtype=bf16)
    hg_maps = []
    for hg in range(2):
        r0, r1 = hg * HCOLS, (hg + 1) * HCOLS
        hg_maps.append({
            "wqt": np.ascontiguousarray(Wq_eff[r0:r1, :].T).astype(bf16),
            "wkt": np.ascontiguousarray(Wk_eff[r0:r1, :].T).astype(bf16),
            "wvt": np.ascontiguousarray(Wv_eff[r0:r1, :].T).astype(bf16),
            "wot": np.ascontiguousarray(Wo_eff[:, r0:r1].T).astype(bf16),
            "ident": ident,
        })
    in_maps = []
    for c in range(NCORES):
        b, hg = c // 2, c % 2
        m = dict(hg_maps[hg])
        m["x"] = np.ascontiguousarray(np.asarray(x, np.float32)[b])
        in_maps.append(m)
    return in_maps


def _run(inputs, trace=False, trace_kwargs=None):
    nc = _get_nc()
    in_maps = make_in_maps(**inputs)
    res = run_bass_kernel_spmd(nc, in_maps, core_ids=list(range(NCORES)),
                               trace=trace, **(trace_kwargs or {}))
    out = np.empty((B, N, DIM), np.float32)
    for b in range(B):
        out[b] = (res.results[2 * b]["out"].astype(np.float32)
                  + res.results[2 * b + 1]["out"].astype(np.float32))
    return out, res


def kernel(x, gamma, Wq, Wk, Wv, Wo):
    out, _ = _run(dict(x=x, gamma=gamma, Wq=Wq, Wk=Wk, Wv=Wv, Wo=Wo))
    return out


# revision 12
# speedup vs baseline: 1.3759x; 1.2260x over previous
"""Fused multi-head attention (LN + QKV + softmax + out-proj) for TRN2,
sharded over 8 NeuronCores: batch (4) x head-group (2 groups of 6 heads).

Per core, for its (batch, head-group) shard:
    xn = LayerNorm(x[b])      (gamma+1 and 1/sqrt(dh) folded into weights)
    Q^T,K^T = W @ xn^T        (head-pair-packed, bf16 matmuls, f32 PSUM)
    V = xn @ Wv               (augmented with a ones column)
    per head, per query-half, per 128-key tile:
        S^T = K^T.T Q^T ; P^T = exp(S^T - C)   (ACT, fp8 out)
        attn^T[65,q] += [V|1].T P^T            (fp8 DoubleRow over key-tile pairs;
                                                row 64 = softmax denominators)
    normalize by row 64, out partial = attn^T.T @ WoT   (bf16 partial out)
Host sums the two partials per batch.
"""
import numpy as np

import concourse.bass as bass
import concourse.bacc as bacc
import concourse.tile as tile
from concourse import mybir
from concourse.bass_utils import run_bass_kernel_spmd

F32 = mybir.dt.float32
BF16 = mybir.dt.bfloat16
FP8 = mybir.dt.float8e4
AF = mybir.ActivationFunctionType
ALU = mybir.AluOpType

B, N, DIM, H, DH = 4, 2048, 768, 12, 64
NCORES = 8
NH = 6            # heads per core
NP = 3            # head pairs per core
HCOLS = NH * DH   # 384
EXP_SHIFT = 3.0   # exp(s - C): keeps fp8 P in range; cancels in softmax

USE_FP8 = False
OUT_BF16 = True


def build_graph(n=N, dim=DIM, num_devices=NCORES, use_fp8=USE_FP8,
                out_bf16=OUT_BF16):
    nt = n // 128        # token/key tiles
    ncdm = dim // 128    # dmodel chunks
    qhw = min(n, 1024)   # query-half width
    nqh = n // qhw

    nc = bacc.Bacc("TRN2", target_bir_lowering=False, debug=False,
                   num_devices=num_devices)
    x = nc.dram_tensor("x", [n, dim], F32, kind="ExternalInput").ap()
    wqt = nc.dram_tensor("wqt", [dim, HCOLS], BF16, kind="ExternalInput").ap()
    wkt = nc.dram_tensor("wkt", [dim, HCOLS], BF16, kind="ExternalInput").ap()
    wvt = nc.dram_tensor("wvt", [dim, HCOLS], BF16, kind="ExternalInput").ap()
    wot = nc.dram_tensor("wot", [HCOLS, dim], BF16, kind="ExternalInput").ap()
    ident = nc.dram_tensor("ident", [128, 128], BF16, kind="ExternalInput").ap()
    odt = BF16 if out_bf16 else F32
    out = nc.dram_tensor("out", [n, dim], odt, kind="ExternalOutput").ap()

    with tile.TileContext(nc) as tc:
        _body(tc, x, wqt, wkt, wvt, wot, ident, out,
              n, dim, nt, ncdm, qhw, nqh, use_fp8)
    nc.compile()
    return nc


def _body(tc, x, wqt, wkt, wvt, wot, ident, out,
          n, dim, nt, ncdm, qhw, nqh, use_fp8):
    nc = tc.nc
    from contextlib import ExitStack
    pdt = FP8 if use_fp8 else BF16
    with ExitStack() as ctx:
        consts = ctx.enter_context(tc.tile_pool(name="consts", bufs=1))
        sb = ctx.enter_context(tc.tile_pool(name="sb", bufs=1))
        xpool = ctx.enter_context(tc.tile_pool(name="xp", bufs=4))
        small = ctx.enter_context(tc.tile_pool(name="small", bufs=4))
        ppool = ctx.enter_context(tc.tile_pool(name="pp", bufs=3))
        rbpool = ctx.enter_context(tc.tile_pool(name="rb", bufs=3))
        oddp = ctx.enter_context(tc.tile_pool(name="odd", bufs=2))
        otp = ctx.enter_context(tc.tile_pool(name="ot", bufs=4))

        # constants + weights
        eps_sb = consts.tile([128, 1], F32, tag="eps")
        nc.vector.memset(eps_sb, 1e-5)
        id_sb = consts.tile([128, 128], BF16, tag="id")
        nc.sync.dma_start(out=id_sb, in_=ident)
        wq_sb = consts.tile([128, ncdm, HCOLS], BF16, tag="wq")
        nc.sync.dma_start(out=wq_sb, in_=wqt.rearrange("(c p) m -> p c m", p=128))
        wk_sb = consts.tile([128, ncdm, HCOLS], BF16, tag="wk")
        nc.sync.dma_start(out=wk_sb, in_=wkt.rearrange("(c p) m -> p c m", p=128))
        wv_sb = consts.tile([128, ncdm, HCOLS], BF16, tag="wv")
        nc.sync.dma_start(out=wv_sb, in_=wvt.rearrange("(c p) m -> p c m", p=128))
        wo_sb = consts.tile([128, NP, dim], BF16, tag="wo")
        nc.sync.dma_start(out=wo_sb, in_=wot.rearrange("(c p) m -> p c m", p=128))

        # persistent activations. Q^T/K^T are stored per head in full-128-row
        # tiles: even heads occupy partitions 0-63 (64-127 zeroed), odd heads
        # occupy 64-127 (0-63 zeroed), so score matmuls contract over K=128
        # with no PE row-tiling mode switches.
        xnT = sb.tile([128, ncdm, n], BF16, tag="xnT")
        qt_sb = sb.tile([128, NH, n], BF16, tag="qt")
        kt_sb = sb.tile([128, NH, n], BF16, tag="kt")
        for hh in range(NH):
            z0 = 64 if hh % 2 == 0 else 0
            nc.gpsimd.memset(qt_sb[z0:z0 + 64, hh, :], 0.0)
            nc.gpsimd.memset(kt_sb[z0:z0 + 64, hh, :], 0.0)
        if use_fp8:
            v_sb = sb.tile([128, NH, nt // 2, 2, DH + 1], pdt, tag="v")
            nc.vector.memset(v_sb[:, :, :, :, DH:DH + 1], 1.0)
        else:
            v_sb = sb.tile([128, NH, nt, DH + 1], pdt, tag="v")
            nc.vector.memset(v_sb[:, :, :, DH:DH + 1], 1.0)
        att_sb = sb.tile([128, NP, n], BF16, tag="att")

        x3 = x.rearrange("(t p) d -> t p d", p=128)
        out3 = out.rearrange("(t p) d -> t p d", p=128)

        # ---- phase 1: LayerNorm + transpose + Q/K/V projections ----
        with tc.tile_pool(name="psA", bufs=6, space="PSUM") as psA:
            for tt in range(nt):
                xt = xpool.tile([128, dim], F32, tag="x")
                nc.sync.dma_start(out=xt, in_=x3[tt])
                ngr = dim // 256
                stats = small.tile([128, ngr, 6], F32, tag="stats")
                for g in range(ngr):
                    nc.vector.bn_stats(out=stats[:, g, :],
                                       in_=xt[:, g * 256:(g + 1) * 256])
                mv = small.tile([128, 2], F32, tag="mv")
                nc.vector.bn_aggr(out=mv, in_=stats)
                sq = small.tile([128, 1], F32, tag="sq")
                nc.scalar.activation(out=sq, in_=mv[:, 1:2], func=AF.Sqrt,
                                     bias=eps_sb)
                rstd = small.tile([128, 1], F32, tag="rstd")
                nc.vector.reciprocal(out=rstd, in_=sq)
                xn = xpool.tile([128, dim], BF16, tag="xn")
                nc.vector.tensor_scalar(out=xn, in0=xt, scalar1=mv[:, 0:1],
                                        scalar2=rstd, op0=ALU.subtract,
                                        op1=ALU.mult)
                for c in range(ncdm):
                    pt = psA.tile([128, 128], BF16, tag="psA")
                    nc.tensor.transpose(pt, xn[:, c * 128:(c + 1) * 128], id_sb)
                    nc.vector.tensor_copy(out=xnT[:, c, tt * 128:(tt + 1) * 128],
                                          in_=pt)

            for i in range(NP):
                for wsb, dst in ((wq_sb, qt_sb), (wk_sb, kt_sb)):
                    for cc in range(n // 512):
                        pst = psA.tile([128, 512], F32, tag="psA")
                        for c in range(ncdm):
                            nc.tensor.matmul(pst, wsb[:, c, i * 128:(i + 1) * 128],
                                             xnT[:, c, cc * 512:(cc + 1) * 512],
                                             start=(c == 0), stop=(c == ncdm - 1))
                        csl = slice(cc * 512, (cc + 1) * 512)
                        nc.vector.tensor_copy(out=dst[0:64, 2 * i, csl],
                                              in_=pst[0:64, :])
                        nc.scalar.copy(out=dst[64:128, 2 * i + 1, csl],
                                       in_=pst[64:128, :])
                for tt in range(nt):
                    pst = psA.tile([128, 128], F32, tag="psA")
                    for c in range(ncdm):
                        nc.tensor.matmul(pst, xnT[:, c, tt * 128:(tt + 1) * 128],
                                         wv_sb[:, c, i * 128:(i + 1) * 128],
                                         start=(c == 0), stop=(c == ncdm - 1))
                    if use_fp8:
                        va = v_sb[:, 2 * i, tt // 2, tt % 2, 0:DH]
                        vb = v_sb[:, 2 * i + 1, tt // 2, tt % 2, 0:DH]
                    else:
                        va = v_sb[:, 2 * i, tt, 0:DH]
                        vb = v_sb[:, 2 * i + 1, tt, 0:DH]
                    nc.vector.tensor_copy(out=va, in_=pst[:, 0:DH])
                    nc.vector.tensor_copy(out=vb, in_=pst[:, DH:2 * DH])

        # ---- phase 2: attention ----
        with tc.tile_pool(name="psS", bufs=2, space="PSUM") as psS, \
             tc.tile_pool(name="psV", bufs=2, space="PSUM") as psV:
            for h in range(NH):
                i, s = h // 2, h % 2
                for qh in range(nqh):
                    q0 = qh * qhw
                    pv = psV.tile([65, qhw], F32, tag="pv")
                    for kt in range(nt):
                        p_t = ppool.tile([128, qhw], pdt, tag="p")
                        sc = psS.tile([128, qhw], F32, tag="sc")
                        for qq in range(qhw // 512):
                            nc.tensor.matmul(
                                sc[:, qq * 512:(qq + 1) * 512],
                                kt_sb[:, h, kt * 128:(kt + 1) * 128],
                                qt_sb[:, h, q0 + qq * 512:q0 + (qq + 1) * 512])
                        nc.scalar.activation(out=p_t, in_=sc, func=AF.Exp)
                        for qq in range(qhw // 512):
                            nc.tensor.matmul(
                                pv[:, qq * 512:(qq + 1) * 512],
                                v_sb[:, h, kt, :],
                                p_t[:, qq * 512:(qq + 1) * 512],
                                start=(kt == 0), stop=(kt == nt - 1))
                    # normalize by softmax denominators (row 64)
                    srow = rbpool.tile([1, qhw], F32, tag="srow")
                    nc.vector.tensor_copy(out=srow, in_=pv[64:65, :])
                    rrow = rbpool.tile([1, qhw], F32, tag="srow")
                    nc.vector.reciprocal_approx_fast(out=rrow, in_=srow)
                    rc = rbpool.tile([64, qhw], F32, tag="rb")
                    nc.gpsimd.partition_broadcast(rc, rrow)
                    if s == 0:
                        nc.vector.tensor_mul(out=att_sb[0:64, i, q0:q0 + qhw],
                                             in0=pv[0:64, :], in1=rc)
                    else:
                        tmp = oddp.tile([64, qhw], BF16, tag="odd")
                        nc.vector.tensor_mul(out=tmp, in0=pv[0:64, :], in1=rc)
                        nc.sync.dma_start(out=att_sb[64:128, i, q0:q0 + qhw],
                                          in_=tmp)

        # ---- phase 3: output projection ----
        with tc.tile_pool(name="psO", bufs=3, space="PSUM") as psO:
            for tt in range(nt):
                po_t = psO.tile([128, dim], F32, tag="psO")
                for c in range(NP):
                    lhsT = att_sb[:, c, tt * 128:(tt + 1) * 128]
                    for o0 in range(0, dim, 512):
                        o1 = min(o0 + 512, dim)
                        nc.tensor.matmul(po_t[:, o0:o1], lhsT, wo_sb[:, c, o0:o1],
                                         start=(c == 0), stop=(c == NP - 1))
                ot = otp.tile([128, dim], out.dtype, tag="ot")
                if tt % 2 == 0:
                    nc.vector.tensor_copy(out=ot, in_=po_t)
                else:
                    nc.scalar.copy(out=ot, in_=po_t)
                nc.sync.dma_start(out=out3[tt], in_=ot)


_NC_CACHE = {}


def _get_nc():
    if "nc" not in _NC_CACHE:
        _NC_CACHE["nc"] = build_graph()
    return _NC_CACHE["nc"]


def make_in_maps(x, gamma, Wq, Wk, Wv, Wo):
    """Host-side sharding: core c -> batch c//2, head-group c%2."""
    import ml_dtypes
    bf16 = ml_dtypes.bfloat16
    g = (np.asarray(gamma, np.float32) + 1.0)
    scale = DH ** -0.5
    Wq_eff = np.asarray(Wq, np.float32) * g[None, :] * scale
    Wk_eff = np.asarray(Wk, np.float32) * g[None, :]
    Wv_eff = np.asarray(Wv, np.float32)
    Wo_eff = np.asarray(Wo, np.float32)
    ident = np.eye(128, dtype=bf16)
    hg_maps = []
    for hg in range(2):
        r0, r1 = hg * HCOLS, (hg + 1) * HCOLS
        hg_maps.append({
            "wqt": np.ascontiguousarray(Wq_eff[r0:r1, :].T).astype(bf16),
            "wkt": np.ascontiguousarray(Wk_eff[r0:r1, :].T).astype(bf16),
            "wvt": np.ascontiguousarray(Wv_eff[r0:r1, :].T).astype(bf16),
            "wot": np.ascontiguousarray(Wo_eff[:, r0:r1].T).astype(bf16),
            "ident": ident,
        })
    in_maps = []
    for c in range(NCORES):
        b, hg = c // 2, c % 2
        m = dict(hg_maps[hg])
        m["x"] = np.ascontiguousarray(np.asarray(x, np.float32)[b])
        in_maps.append(m)
    return in_maps


def _run(inputs, trace=False, trace_kwargs=None):
    nc = _get_nc()
    in_maps = make_in_maps(**inputs)
    res = run_bass_kernel_spmd(nc, in_maps, core_ids=list(range(NCORES)),
                               trace=trace, **(trace_kwargs or {}))
    out = np.empty((B, N, DIM), np.float32)
    for b in range(B):
        out[b] = (res.results[2 * b]["out"].astype(np.float32)
                  + res.results[2 * b + 1]["out"].astype(np.float32))
    return out, res


def kernel(x, gamma, Wq, Wk, Wv, Wo):
    out, _ = _run(dict(x=x, gamma=gamma, Wq=Wq, Wk=Wk, Wv=Wv, Wo=Wo))
    return out
